# revision 12
# baseline (speedup 1.0000x reference)
"""Bahdanau attention kernel for Trainium2 (8 NeuronCores, data-parallel over batch).

reference:
    wh      = einsum('bsh,oh->bso', encoder_outputs, W_h)          # [B,S,H]
    ws      = einsum('bh,oh->bo',  decoder_hidden,  W_s)[:,None,:] # [B,1,H]
    energy  = tanh(wh + ws)                                        # [B,S,H]
    scores  = einsum('bso,o->bs', energy, v)                       # [B,S]
    attn    = softmax(scores, axis=1)                              # [B,S]
    context = einsum('bs,bsh->bh', attn, encoder_outputs)          # [B,H]
    return (context, attn)

B=32, S=2048, H=1024.  Shard batch 4-per-core across 8 cores; weights replicated.

Per-core plan (all loops fully unrolled under Tile):
  prep: cast W_h/W_s/enc fp32->bf16 into DRAM scratch (SWDGE cast DMA), then
        DMA-xbar-transpose loads into SBUF:
          WT  [h_in(p), h_out]  (lhsT layout for PE)
          ET  [h(p), s]         per batch (rhs layout; also feeds context)
  wsT = W_s @ dec^T on PE -> [h_out(p), b] (bias for tanh)
  main per (b, o-chunk): PE accumulates whT chunk [128, S] over 8 k-chunks into
        PSUM; ScalarE applies tanh(. + wsT bias) -> energy bf16; PE dots with v
        chunk accumulating scores [1, S] over o-chunks in PSUM.
  softmax on [1, S]: DVE max, ScalarE exp (bias=-max, accum_out=sum), DVE
        reciprocal + scale.
  context: gpsimd partition_broadcast of attn to 128 partitions, then per
        h-chunk one fused DVE tensor_tensor_reduce (mult, add) against ET.
"""

import sys

if "/opt/trn_rl_repo" not in sys.path:
    sys.path.insert(0, "/opt/trn_rl_repo")

from contextlib import ExitStack

import numpy as np

import concourse.bacc as bacc
import concourse.bass as bass
import concourse.mybir as mybir
import concourse.tile as tile
from concourse.bass import ds, ts
from concourse.bass_utils import run_bass_kernel_spmd

B, S, H = 32, 2048, 1024
NCORES = 8
BL = B // NCORES  # batches per core
P = 128
KC = H // P  # contraction chunks
OC = H // P  # output-feature chunks
NS = 512  # matmul moving free dim (one PSUM bank of fp32)
SC = S // NS

f32 = mybir.dt.float32
bf16 = mybir.dt.bfloat16

# Debug: 1=prep only, 2=+matmul/tanh, 3=+scores/softmax, 4=full (default)
STAGE = 4
AF = mybir.ActivationFunctionType
ALU = mybir.AluOpType
AX = mybir.AxisListType


def _kernel_body(ectx: ExitStack, tc, dec, enc, w_h, w_s, v, ctx_out, attn_out,
                 enc_bf, w_h_bf, w_s_bf):
    nc = tc.nc

    const = ectx.enter_context(tc.tile_pool(name="const", bufs=1))
    etp = ectx.enter_context(tc.tile_pool(name="etp", bufs=2))
    enp = ectx.enter_context(tc.tile_pool(name="enp", bufs=2))
    small = ectx.enter_context(tc.tile_pool(name="small", bufs=1))
    rot = ectx.enter_context(tc.tile_pool(name="rot", bufs=2))
    psmm = ectx.enter_context(tc.tile_pool(name="psmm", bufs=2, space="PSUM"))
    pssc = ectx.enter_context(tc.tile_pool(name="pssc", bufs=1, space="PSUM"))

    # ---- casts to bf16 DRAM scratch (SWDGE can cast; HWDGE cannot) ----
    nc.gpsimd.dma_start(out=w_h_bf[:, :], in_=w_h[:, :])
    nc.gpsimd.dma_start(out=w_s_bf[:, :], in_=w_s[:, :])
    for b in range(BL):
        for sc in range(SC):
            nc.gpsimd.dma_start(
                out=enc_bf[b, ts(sc, NS), :], in_=enc[b, ts(sc, NS), :]
            )

    # ---- transposed weight loads: WT[:, k, :] = W[:, k*128:(k+1)*128].T ----
    w_hT = const.tile([P, KC, H], bf16)
    w_sT = const.tile([P, KC, H], bf16)
    for k in range(KC):
        nc.sync.dma_start(out=w_hT[:, k, :], in_=w_h_bf[:, ts(k, P)], transpose=True)
        nc.sync.dma_start(out=w_sT[:, k, :], in_=w_s_bf[:, ts(k, P)], transpose=True)

    # ---- v -> [128, KC] bf16 (chunk k lives in column k) ----
    v_f = const.tile([P, KC], f32)
    nc.sync.dma_start(out=v_f[:, :], in_=v.rearrange("(c p) -> p c", p=P))
    vt = const.tile([P, KC], bf16)
    nc.vector.tensor_copy(vt[:, :], v_f[:, :])

    # ---- dec^T -> [128, KC, BL] bf16 (small strided load, then cast) ----
    dT_f = const.tile([P, KC, BL], f32)
    for k in range(KC):
        nc.sync.dma_start(
            out=dT_f[:, k, :], in_=dec[:, ts(k, P)].rearrange("b h -> h b")
        )
    dT = const.tile([P, KC, BL], bf16)
    nc.vector.tensor_copy(dT[:, :, :], dT_f[:, :, :])

    # ---- wsT[o-chunk] = (W_s @ dec^T)[o-chunk] : [128, BL] fp32 ----
    ws_sb = const.tile([P, OC, BL], f32)
    for o in range(OC):
        ws_ps = psmm.tile([P, BL], f32, tag="mmps")
        for k in range(KC):
            nc.tensor.matmul(
                ws_ps[:, :],
                lhsT=w_sT[:, k, ts(o, P)],
                rhs=dT[:, k, :],
                start=(k == 0),
                stop=(k == KC - 1),
            )
        nc.vector.tensor_copy(ws_sb[:, o, :], ws_ps[:, :])

    # ---- main loop over local batches ----
    for b in range(BL):
        # E^T for this batch: [128, KC, S] bf16 via DMA xbar transpose
        et = etp.tile([P, KC, S], bf16, tag="et")
        for k in range(KC):
            nc.sync.dma_start(
                out=et[:, k, :], in_=enc_bf[b, :, ts(k, P)], transpose=True
            )
        if STAGE < 2:
            if b == 0:
                dbg = rot.tile([P, OC], f32, tag="ctx")
                nc.vector.tensor_copy(dbg[:, :], et[:, 0, 0:OC])
                nc.sync.dma_start(
                    out=ctx_out[b, :].rearrange("(c p) -> p c", p=P),
                    in_=dbg[:, :],
                )
            continue

        scores_ps = [
            pssc.tile([1, NS], f32, tag=f"sc{s4}", name=f"scores_ps{s4}")
            for s4 in range(SC)
        ]

        for o in range(OC):
            energy = enp.tile([P, S], bf16, tag="energy")
            for sh in range(2):  # halves of S, 1024 each (2 PSUM banks)
                wh_ps = psmm.tile([P, 2 * NS], f32, tag="mmps")
                for s4i in range(2):
                    s_off = sh * 2 * NS + s4i * NS
                    for k in range(KC):
                        nc.tensor.matmul(
                            wh_ps[:, ts(s4i, NS)],
                            lhsT=w_hT[:, k, ts(o, P)],
                            rhs=et[:, k, ds(s_off, NS)],
                            start=(k == 0),
                            stop=(k == KC - 1),
                        )
                # energy = tanh(whT + wsT[:, o, b])
                nc.scalar.activation(
                    energy[:, ds(sh * 2 * NS, 2 * NS)],
                    wh_ps[:, :],
                    AF.Tanh,
                    bias=ws_sb[:, o, ds(b, 1)],
                )
            # scores += v[o-chunk] . energy
            if STAGE >= 3:
                for s4 in range(SC):
                    nc.tensor.matmul(
                        scores_ps[s4][:, :],
                        lhsT=vt[:, ds(o, 1)],
                        rhs=energy[:, ts(s4, NS)],
                        start=(o == 0),
                        stop=(o == OC - 1),
                    )
            elif o == OC - 1:
                dbg2 = rot.tile([P, OC], f32, tag="ctx")
                nc.vector.tensor_copy(dbg2[:, :], energy[:, 0:OC])
                nc.sync.dma_start(
                    out=ctx_out[b, :].rearrange("(c p) -> p c", p=P),
                    in_=dbg2[:, :],
                )
        if STAGE < 3:
            continue

        # ---- softmax over s on partition 0 ----
        scores_sb = small.tile([1, S], f32, tag="scores")
        for s4 in range(SC):
            nc.vector.tensor_copy(scores_sb[:, ts(s4, NS)], scores_ps[s4][:, :])
        negmax = small.tile([1, 1], f32, tag="negmax")
        nc.vector.tensor_reduce(
            negmax[:, :], scores_sb[:, :], axis=AX.X, op=ALU.max, negate=True
        )
        exp_sb = small.tile([1, S], f32, tag="exp")
        sumexp = small.tile([1, 1], f32, tag="sumexp")
        nc.scalar.activation(
            exp_sb[:, :], scores_sb[:, :], AF.Exp,
            bias=negmax[:, :], accum_out=sumexp[:, :],
        )
        rsum = small.tile([1, 1], f32, tag="rsum")
        nc.vector.reciprocal(rsum[:, :], sumexp[:, :])
        attn_sb = small.tile([1, S], f32, tag="attn")
        nc.vector.tensor_scalar_mul(attn_sb[:, :], exp_sb[:, :], rsum[:, :])
        nc.sync.dma_start(out=attn_out[b, :], in_=attn_sb[:, :])

        if STAGE < 4:
            continue

        # ---- context: broadcast attn across partitions, fused mul+reduce ----
        # (replicated DRAM->SBUF load: 0-stride partition dim on the source;
        #  avoids gpsimd ucode + the DMA-transpose ‖ SBUF->SBUF hazard)
        attn_rep = rot.tile([P, S], f32, tag="attnrep")
        nc.sync.dma_start(
            out=attn_rep[:, :],
            in_=attn_out[b, :].unsqueeze(0).broadcast_to([P, S]),
        )
        ctx_sb = rot.tile([P, OC], f32, tag="ctx")
        for o in range(OC):
            # DVE multiply, then ScalarE Copy with accum_out for the s-sum
            # (tensor_tensor_reduce is broken on HW)
            prod = rot.tile([P, S], f32, tag="prod")
            nc.vector.tensor_mul(prod[:, :], et[:, o, :], attn_rep[:, :])
            junk = rot.tile([P, S], bf16, tag="junk")
            nc.scalar.activation(
                junk[:, :], prod[:, :], AF.Copy,
                accum_out=ctx_sb[:, ds(o, 1)],
            )
        nc.sync.dma_start(
            out=ctx_out[b, :].rearrange("(c p) -> p c", p=P), in_=ctx_sb[:, :]
        )


def build_nc():
    nc = bacc.Bacc(
        "TRN2", target_bir_lowering=False, debug=False, num_devices=NCORES
    )
    dec = nc.dram_tensor("dec", [BL, H], f32, kind="ExternalInput").ap()
    enc = nc.dram_tensor("enc", [BL, S, H], f32, kind="ExternalInput").ap()
    w_h = nc.dram_tensor("w_h", [H, H], f32, kind="ExternalInput").ap()
    w_s = nc.dram_tensor("w_s", [H, H], f32, kind="ExternalInput").ap()
    v = nc.dram_tensor("v", [H], f32, kind="ExternalInput").ap()
    ctx_out = nc.dram_tensor("ctx_out", [BL, H], f32, kind="ExternalOutput").ap()
    attn_out = nc.dram_tensor("attn_out", [BL, S], f32, kind="ExternalOutput").ap()
    enc_bf = nc.dram_tensor("enc_bf", [BL, S, H], bf16, kind="Internal").ap()
    w_h_bf = nc.dram_tensor("w_h_bf", [H, H], bf16, kind="Internal").ap()
    w_s_bf = nc.dram_tensor("w_s_bf", [H, H], bf16, kind="Internal").ap()

    with tile.TileContext(nc) as tc:
        with ExitStack() as ectx:
            _kernel_body(ectx, tc, dec, enc, w_h, w_s, v, ctx_out, attn_out,
                         enc_bf, w_h_bf, w_s_bf)
    nc.compile()
    return nc


_NC_CACHE = None


def _get_nc():
    global _NC_CACHE
    if _NC_CACHE is None:
        _NC_CACHE = build_nc()
    return _NC_CACHE


def _in_maps(decoder_hidden, encoder_outputs, W_h, W_s, v):
    dec = np.ascontiguousarray(np.asarray(decoder_hidden, dtype=np.float32))
    enc = np.ascontiguousarray(np.asarray(encoder_outputs, dtype=np.float32))
    w_h = np.ascontiguousarray(np.asarray(W_h, dtype=np.float32))
    w_s = np.ascontiguousarray(np.asarray(W_s, dtype=np.float32))
    vv = np.ascontiguousarray(np.asarray(v, dtype=np.float32))
    maps = []
    for c in range(NCORES):
        sl = slice(c * BL, (c + 1) * BL)
        maps.append(
            {"dec": dec[sl], "enc": enc[sl], "w_h": w_h, "w_s": w_s, "v": vv}
        )
    return maps


def run(decoder_hidden, encoder_outputs, W_h, W_s, v, trace=False, **trace_kw):
    nc = _get_nc()
    res = run_bass_kernel_spmd(
        nc,
        _in_maps(decoder_hidden, encoder_outputs, W_h, W_s, v),
        list(range(NCORES)),
        trace=trace,
        **trace_kw,
    )
    ctx = np.concatenate([r["ctx_out"] for r in res.results], axis=0)
    attn = np.concatenate([r["attn_out"] for r in res.results], axis=0)
    return (ctx.astype(np.float32), attn.astype(np.float32)), res


def kernel(decoder_hidden, encoder_outputs, W_h, W_s, v):
    (ctx, attn), _ = run(decoder_hidden, encoder_outputs, W_h, W_s, v)
    return (ctx, attn)


# revision 15
# speedup vs baseline: 1.0034x; 1.0034x over previous
"""Bahdanau attention kernel for Trainium2 (8 NeuronCores, data-parallel over batch).

reference:
    wh      = einsum('bsh,oh->bso', encoder_outputs, W_h)          # [B,S,H]
    ws      = einsum('bh,oh->bo',  decoder_hidden,  W_s)[:,None,:] # [B,1,H]
    energy  = tanh(wh + ws)                                        # [B,S,H]
    scores  = einsum('bso,o->bs', energy, v)                       # [B,S]
    attn    = softmax(scores, axis=1)                              # [B,S]
    context = einsum('bs,bsh->bh', attn, encoder_outputs)          # [B,H]
    return (context, attn)

B=32, S=2048, H=1024.  Shard batch 4-per-core across 8 cores; weights replicated.

Per-core plan (all loops fully unrolled under Tile):
  prep: cast W_h/W_s/enc fp32->bf16 into DRAM scratch (SWDGE cast DMA), then
        DMA-xbar-transpose loads into SBUF:
          WT  [h_in(p), h_out]  (lhsT layout for PE)
          ET  [h(p), s]         per batch (rhs layout; also feeds context)
  wsT = W_s @ dec^T on PE -> [h_out(p), b] (bias for tanh)
  main per (b, o-chunk): PE accumulates whT chunk [128, S] over 8 k-chunks into
        PSUM; ScalarE applies tanh(. + wsT bias) -> energy bf16; PE dots with v
        chunk accumulating scores [1, S] over o-chunks in PSUM.
  softmax on [1, S]: DVE max, ScalarE exp (bias=-max, accum_out=sum), DVE
        reciprocal + scale.
  context: gpsimd partition_broadcast of attn to 128 partitions, then per
        h-chunk one fused DVE tensor_tensor_reduce (mult, add) against ET.
"""

import sys

if "/opt/trn_rl_repo" not in sys.path:
    sys.path.insert(0, "/opt/trn_rl_repo")

from contextlib import ExitStack

import numpy as np

import concourse.bacc as bacc
import concourse.bass as bass
import concourse.mybir as mybir
import concourse.tile as tile
from concourse.bass import ds, ts
from concourse.bass_utils import run_bass_kernel_spmd

B, S, H = 32, 2048, 1024
NCORES = 8
BL = B // NCORES  # batches per core
P = 128
KC = H // P  # contraction chunks
OC = H // P  # output-feature chunks
NS = 512  # matmul moving free dim (one PSUM bank of fp32)
SC = S // NS

f32 = mybir.dt.float32
bf16 = mybir.dt.bfloat16

# Debug: 1=prep only, 2=+matmul/tanh, 3=+scores/softmax, 4=full (default)
STAGE = 4
AF = mybir.ActivationFunctionType
ALU = mybir.AluOpType
AX = mybir.AxisListType


def _kernel_body(ectx: ExitStack, tc, dec, enc, w_h, w_s, v, ctx_out, attn_out,
                 enc_bf, w_h_bf, w_s_bf):
    nc = tc.nc

    const = ectx.enter_context(tc.tile_pool(name="const", bufs=1))
    etp = ectx.enter_context(tc.tile_pool(name="etp", bufs=2))
    enp = ectx.enter_context(tc.tile_pool(name="enp", bufs=2))
    small = ectx.enter_context(tc.tile_pool(name="small", bufs=1))
    rot = ectx.enter_context(tc.tile_pool(name="rot", bufs=2))
    psmm = ectx.enter_context(tc.tile_pool(name="psmm", bufs=2, space="PSUM"))
    pssc = ectx.enter_context(tc.tile_pool(name="pssc", bufs=1, space="PSUM"))

    # ---- casts to bf16 DRAM scratch (SWDGE can cast; HWDGE cannot) ----
    # Order matters on the gpsimd FIFO: W_s first (ws bias is needed by the
    # first tanh), then W_h, then enc halves batch-by-batch.
    nc.gpsimd.dma_start(out=w_s_bf[:, :], in_=w_s[:, :])
    nc.gpsimd.dma_start(out=w_h_bf[:, :], in_=w_h[:, :])
    SH = S // 2
    for b in range(BL):
        for sh in range(2):
            nc.gpsimd.dma_start(
                out=enc_bf[b, ts(sh, SH), :], in_=enc[b, ts(sh, SH), :]
            )

    # ---- prep loads on the scalar HWDGE ring (keeps the sync ring free
    #      for the enc transposes that gate the first matmuls) ----
    # dec^T (strided, slow descriptors) first: only needs DRAM inputs.
    dT_f = const.tile([P, KC, BL], f32)
    for k in range(KC):
        nc.scalar.dma_start(
            out=dT_f[:, k, :], in_=dec[:, ts(k, P)].rearrange("b h -> h b")
        )
    dT = const.tile([P, KC, BL], bf16)
    nc.vector.tensor_copy(dT[:, :, :], dT_f[:, :, :])

    # transposed weight loads: WT[:, k, :] = W[:, k*128:(k+1)*128].T
    w_sT = const.tile([P, KC, H], bf16)
    for k in range(KC):
        nc.scalar.dma_start(
            out=w_sT[:, k, :], in_=w_s_bf[:, ts(k, P)], transpose=True
        )
    v_f = const.tile([P, KC], f32)
    nc.scalar.dma_start(out=v_f[:, :], in_=v.rearrange("(c p) -> p c", p=P))
    vt = const.tile([P, KC], bf16)
    nc.vector.tensor_copy(vt[:, :], v_f[:, :])
    w_hT = const.tile([P, KC, H], bf16)
    for k in range(KC):
        nc.scalar.dma_start(
            out=w_hT[:, k, :], in_=w_h_bf[:, ts(k, P)], transpose=True
        )

    # ---- wsT[o-chunk] = (W_s @ dec^T)[o-chunk] : [128, BL] fp32 ----
    ws_sb = const.tile([P, OC, BL], f32)
    for o in range(OC):
        ws_ps = psmm.tile([P, BL], f32, tag="mmps")
        for k in range(KC):
            nc.tensor.matmul(
                ws_ps[:, :],
                lhsT=w_sT[:, k, ts(o, P)],
                rhs=dT[:, k, :],
                start=(k == 0),
                stop=(k == KC - 1),
            )
        nc.vector.tensor_copy(ws_sb[:, o, :], ws_ps[:, :])

    # ---- main loop over local batches ----
    for b in range(BL):
        # E^T for this batch: [128, KC, S] bf16 via DMA xbar transpose,
        # per (k, s-half) so the first matmuls only wait on half a batch
        et = etp.tile([P, KC, S], bf16, tag="et")
        for sh in range(2):
            for k in range(KC):
                nc.sync.dma_start(
                    out=et[:, k, ts(sh, SH)],
                    in_=enc_bf[b, ts(sh, SH), ts(k, P)],
                    transpose=True,
                )
        if STAGE < 2:
            if b == 0:
                dbg = rot.tile([P, OC], f32, tag="ctx")
                nc.vector.tensor_copy(dbg[:, :], et[:, 0, 0:OC])
                nc.sync.dma_start(
                    out=ctx_out[b, :].rearrange("(c p) -> p c", p=P),
                    in_=dbg[:, :],
                )
            continue

        scores_ps = [
            pssc.tile([1, NS], f32, tag=f"sc{s4}", name=f"scores_ps{s4}")
            for s4 in range(SC)
        ]

        for o in range(OC):
            energy = enp.tile([P, S], bf16, tag="energy")
            for sh in range(2):  # halves of S, 1024 each (2 PSUM banks)
                wh_ps = psmm.tile([P, 2 * NS], f32, tag="mmps")
                # k outer: one weight load feeds both 512-wide moving tiles
                for k in range(KC):
                    for s4i in range(2):
                        s_off = sh * 2 * NS + s4i * NS
                        nc.tensor.matmul(
                            wh_ps[:, ts(s4i, NS)],
                            lhsT=w_hT[:, k, ts(o, P)],
                            rhs=et[:, k, ds(s_off, NS)],
                            start=(k == 0),
                            stop=(k == KC - 1),
                        )
                # energy = tanh(whT + wsT[:, o, b])
                nc.scalar.activation(
                    energy[:, ds(sh * 2 * NS, 2 * NS)],
                    wh_ps[:, :],
                    AF.Tanh,
                    bias=ws_sb[:, o, ds(b, 1)],
                )
            # scores += v[o-chunk] . energy
            if STAGE >= 3:
                for s4 in range(SC):
                    nc.tensor.matmul(
                        scores_ps[s4][:, :],
                        lhsT=vt[:, ds(o, 1)],
                        rhs=energy[:, ts(s4, NS)],
                        start=(o == 0),
                        stop=(o == OC - 1),
                    )
            elif o == OC - 1:
                dbg2 = rot.tile([P, OC], f32, tag="ctx")
                nc.vector.tensor_copy(dbg2[:, :], energy[:, 0:OC])
                nc.sync.dma_start(
                    out=ctx_out[b, :].rearrange("(c p) -> p c", p=P),
                    in_=dbg2[:, :],
                )
        if STAGE < 3:
            continue

        # ---- softmax over s on partition 0 ----
        scores_sb = small.tile([1, S], f32, tag="scores")
        for s4 in range(SC):
            nc.vector.tensor_copy(scores_sb[:, ts(s4, NS)], scores_ps[s4][:, :])
        negmax = small.tile([1, 1], f32, tag="negmax")
        nc.vector.tensor_reduce(
            negmax[:, :], scores_sb[:, :], axis=AX.X, op=ALU.max, negate=True
        )
        exp_sb = small.tile([1, S], f32, tag="exp")
        sumexp = small.tile([1, 1], f32, tag="sumexp")
        nc.scalar.activation(
            exp_sb[:, :], scores_sb[:, :], AF.Exp,
            bias=negmax[:, :], accum_out=sumexp[:, :],
        )
        rsum = small.tile([1, 1], f32, tag="rsum")
        nc.vector.reciprocal(rsum[:, :], sumexp[:, :])
        attn_sb = small.tile([1, S], f32, tag="attn")
        nc.vector.tensor_scalar_mul(attn_sb[:, :], exp_sb[:, :], rsum[:, :])
        nc.sync.dma_start(out=attn_out[b, :], in_=attn_sb[:, :])

        if STAGE < 4:
            continue

        # ---- context: broadcast attn across partitions, fused mul+reduce ----
        # (replicated DRAM->SBUF load: 0-stride partition dim on the source;
        #  avoids gpsimd ucode + the DMA-transpose ‖ SBUF->SBUF hazard)
        attn_rep = rot.tile([P, S], f32, tag="attnrep")
        nc.sync.dma_start(
            out=attn_rep[:, :],
            in_=attn_out[b, :].unsqueeze(0).broadcast_to([P, S]),
        )
        ctx_sb = rot.tile([P, OC], f32, tag="ctx")
        for o in range(OC):
            # DVE multiply, then ScalarE Copy with accum_out for the s-sum
            # (tensor_tensor_reduce is broken on HW)
            prod = rot.tile([P, S], f32, tag="prod")
            nc.vector.tensor_mul(prod[:, :], et[:, o, :], attn_rep[:, :])
            junk = rot.tile([P, S], bf16, tag="junk")
            nc.scalar.activation(
                junk[:, :], prod[:, :], AF.Copy,
                accum_out=ctx_sb[:, ds(o, 1)],
            )
        nc.sync.dma_start(
            out=ctx_out[b, :].rearrange("(c p) -> p c", p=P), in_=ctx_sb[:, :]
        )


def build_nc():
    nc = bacc.Bacc(
        "TRN2", target_bir_lowering=False, debug=False, num_devices=NCORES
    )
    dec = nc.dram_tensor("dec", [BL, H], f32, kind="ExternalInput").ap()
    enc = nc.dram_tensor("enc", [BL, S, H], f32, kind="ExternalInput").ap()
    w_h = nc.dram_tensor("w_h", [H, H], f32, kind="ExternalInput").ap()
    w_s = nc.dram_tensor("w_s", [H, H], f32, kind="ExternalInput").ap()
    v = nc.dram_tensor("v", [H], f32, kind="ExternalInput").ap()
    ctx_out = nc.dram_tensor("ctx_out", [BL, H], f32, kind="ExternalOutput").ap()
    attn_out = nc.dram_tensor("attn_out", [BL, S], f32, kind="ExternalOutput").ap()
    enc_bf = nc.dram_tensor("enc_bf", [BL, S, H], bf16, kind="Internal").ap()
    w_h_bf = nc.dram_tensor("w_h_bf", [H, H], bf16, kind="Internal").ap()
    w_s_bf = nc.dram_tensor("w_s_bf", [H, H], bf16, kind="Internal").ap()

    with tile.TileContext(nc) as tc:
        with ExitStack() as ectx:
            _kernel_body(ectx, tc, dec, enc, w_h, w_s, v, ctx_out, attn_out,
                         enc_bf, w_h_bf, w_s_bf)
    nc.compile()
    return nc


_NC_CACHE = None


def _get_nc():
    global _NC_CACHE
    if _NC_CACHE is None:
        _NC_CACHE = build_nc()
    return _NC_CACHE


def _in_maps(decoder_hidden, encoder_outputs, W_h, W_s, v):
    dec = np.ascontiguousarray(np.asarray(decoder_hidden, dtype=np.float32))
    enc = np.ascontiguousarray(np.asarray(encoder_outputs, dtype=np.float32))
    w_h = np.ascontiguousarray(np.asarray(W_h, dtype=np.float32))
    w_s = np.ascontiguousarray(np.asarray(W_s, dtype=np.float32))
    vv = np.ascontiguousarray(np.asarray(v, dtype=np.float32))
    maps = []
    for c in range(NCORES):
        sl = slice(c * BL, (c + 1) * BL)
        maps.append(
            {"dec": dec[sl], "enc": enc[sl], "w_h": w_h, "w_s": w_s, "v": vv}
        )
    return maps


def run(decoder_hidden, encoder_outputs, W_h, W_s, v, trace=False, **trace_kw):
    nc = _get_nc()
    res = run_bass_kernel_spmd(
        nc,
        _in_maps(decoder_hidden, encoder_outputs, W_h, W_s, v),
        list(range(NCORES)),
        trace=trace,
        **trace_kw,
    )
    ctx = np.concatenate([r["ctx_out"] for r in res.results], axis=0)
    attn = np.concatenate([r["attn_out"] for r in res.results], axis=0)
    return (ctx.astype(np.float32), attn.astype(np.float32)), res


def kernel(decoder_hidden, encoder_outputs, W_h, W_s, v):
    (ctx, attn), _ = run(decoder_hidden, encoder_outputs, W_h, W_s, v)
    return (ctx, attn)
